# revision 18
# baseline (speedup 1.0000x reference)
"""Grouped SwiGLU expert MLP (MoE) on 8 Trainium2 NeuronCores.

Problem: sorted_x [32768, 512] f32, tokens pre-sorted by expert into 8 equal
contiguous segments of 4096 tokens; per-expert SwiGLU MLP
    h12 = x_e @ w12[e].T          (4096, 2816)
    h   = silu(h12[:, :1408]) * h12[:, 1408:]
    out = h @ w3[e].T             (4096, 512)

Sharding: pure expert parallelism — core e owns expert e's weights and its
4096-token segment (sliced host-side from expert_starts), so no device-side
collectives are needed; the host concatenates the per-core outputs.

Device layout is feature-major throughout ("contraction dim on partitions"),
which makes both GEMMs transpose-free on chip:
    xt   = x_e.T   [512, 4096]  fp16
    w12t = w12.T   [512, 2816]  fp16
    w3t  = w3.T    [1408, 512]  fp16
    outT = out.T   [512, 4096]  f32   (host transposes back)
GEMM1 produces H12^T tiles [128h, Nt] (PSUM), SwiGLU runs on ACT+DVE into
fp16 H^T tiles, GEMM2 consumes them directly. fp16 operands run the PE at
1 cycle/row (vs 4 for f32) — same speed and footprint as bf16 with a 10-bit
mantissa (8x lower rounding error; inputs here are well inside fp16 range).
Accumulation is always f32 in PSUM.
"""

import os

import numpy as np
import ml_dtypes

import concourse.bass as bass
import concourse.mybir as mybir
import concourse.tile as tile
from concourse import bacc
from concourse.bass_utils import run_bass_kernel_spmd

N_CORES = 8
D = 512  # d_model
H = 1408  # hidden
TWOH = 2 * H
TPE = 4096  # tokens per expert
NT = 512  # token block (matmul moving free dim, one PSUM bank in f32)
KD = D // 128  # 4 contraction tiles over d
KH = H // 128  # 11 contraction tiles over h
NB = TPE // NT  # token blocks

F16 = mybir.dt.float16
F32 = mybir.dt.float32
NP_F16 = np.dtype(np.float16)

# Results of a traced run (test harness reads these).
last_exec_time_ns = None
last_trace_path = None


def _build():
    # Bacc (not plain Bass): its compile() pass pipeline legalizes sync
    # waits (>=2 waits per instruction are split into event-sem chains),
    # which this image's walrus requires.
    nc = bacc.Bacc("TRN2", target_bir_lowering=False, debug=False, num_devices=N_CORES)
    xt = nc.dram_tensor("xt", [D, TPE], F16, kind="ExternalInput")
    w12t = nc.dram_tensor("w12t", [D, TWOH], F16, kind="ExternalInput")
    w3t = nc.dram_tensor("w3t", [H, D], F16, kind="ExternalInput")
    outT = nc.dram_tensor("outT", [D, TPE], F32, kind="ExternalOutput")

    # GEMM2 is software-pipelined into the GEMM1/SwiGLU loop with this lag:
    # in iteration hh we issue the GEMM2 matmuls consuming ht[hh - LAG], so
    # the PE never waits on the ACT+DVE SwiGLU chain (~1.3us behind).
    LAG = 3

    # Dummy warm-up matmuls issued while the first input chunks are still in
    # flight: the PE's HAM clock gate needs ~3.4us of sustained activity to
    # lift the cold 4/8 throttle, so burn the DMA head warming it up and the
    # real matmuls start at full rate.  N=128 keeps the per-dummy FIFO delay
    # (if data lands early) down to ~107ns.  25 dummies cover the ~2.7us
    # from sequencer-preamble end to the first 128KB chunks landing; a few
    # more are interleaved into block 0's first hh groups (see loop) so the
    # scattered sub-us chunk-landing stalls can't re-throttle the HAM.
    N_WARM = 38
    WARM_IN = {(0, 0): 2, (0, 1): 2, (1, 0): 1, (1, 1): 1}

    with tile.TileContext(nc) as tc:
        with (
            tc.tile_pool(name="weights", bufs=1) as wpool,
            tc.tile_pool(name="xin", bufs=1) as xpool,
            tc.tile_pool(name="ht", bufs=2) as hpool,
            tc.tile_pool(name="swi", bufs=4) as spool,
            tc.tile_pool(name="ot", bufs=4) as opool,
            tc.tile_pool(name="dmy", bufs=1) as dpool,
            tc.tile_pool(name="pg", bufs=2, space=bass.MemorySpace.PSUM) as pgate,
            tc.tile_pool(name="pu", bufs=2, space=bass.MemorySpace.PSUM) as pup,
            tc.tile_pool(name="po", bufs=1, space=bass.MemorySpace.PSUM) as pacc,
        ):
            w12s = wpool.tile([128, KD, TWOH], F16)
            w3s = wpool.tile([128, KH, D], F16)
            xs = xpool.tile([128, KD, TPE], F16)
            dmy = dpool.tile([128, 128], F16)

            nc.vector.memset(dmy[:], 0.0)
            ps_dummy = pgate.tile([128, NT], F32, tag="ps_g")
            for _ in range(N_WARM):
                nc.tensor.matmul(
                    ps_dummy[:, 0:128], dmy[:], dmy[:], start=True, stop=True
                )

            # Each DMA_DIRECT2D costs ~650ns (HWDGE) / ~900ns (SWDGE) of
            # sequencer issue time, each ring executes its transfers serially
            # in FIFO order, and the 16 SDMA engines round-robin between busy
            # rings at packet granularity (so N busy rings each get ~1/N of
            # the ~358GB/s HBM bandwidth).  Layout: the two rings each carry
            # ~half of the block-0-critical bytes, ordered EXACTLY as the PE
            # consumes them, with the non-critical x blocks 2..7 split across
            # both ring tails:
            #   SP   (qSPDynamicHW): w12 gate/up-interleaved hh-chunks, then
            #                        x2,x4,x6, then per-block output stores
            #   Pool (SWDGE):        x block 0 per-kd (the first matmul only
            #                        needs one 128KB chunk), w3 in gemm2
            #                        order, x1, then x3,x5,x7
            #   ACT  (qActDynamicHW): idle until the last block's stores
            # Deps are per-DMA-instruction, so fine chunks = early starts.
            xt_r = xt[:, :].rearrange("(kd p) t -> p kd t", p=128)
            w12_r = w12t[:, :].rearrange("(kd p) h -> p kd h", p=128)
            w3_r = w3t[:, :].rearrange("(kh p) d -> p kh d", p=128)

            def dma_w12(c0, c1):
                nc.sync.dma_start(out=w12s[:, :, c0:c1], in_=w12_r[:, :, c0:c1])

            def dma_x(tb, eng):
                eng.dma_start(
                    out=xs[:, :, tb * NT : (tb + 1) * NT],
                    in_=xt_r[:, :, tb * NT : (tb + 1) * NT],
                )

            def dma_x0(kd, eng):
                eng.dma_start(out=xs[:, kd, 0:NT], in_=xt_r[:, kd, 0:NT])

            def dma_w3(k0, k1):
                nc.gpsimd.dma_start(out=w3s[:, k0:k1, :], in_=w3_r[:, k0:k1, :])

            def dma_w12_e(c0, c1, eng):
                eng.dma_start(out=w12s[:, :, c0:c1], in_=w12_r[:, :, c0:c1])

            # Three rings each carry ~1/3 of the block-0-critical bytes in
            # their consumers' order (HWDGE rings backpressure the issuing
            # sequencer past ~9 outstanding DMAs, so each ring gets few,
            # need-ordered chunks; x tails ride the SWDGE ring, whose
            # descriptor rings absorb deep queues).
            # sync ring: gate-half of w12 head + x0 kd1, then stores.
            # 5 DMAs ~= 2.2K descriptors stays under the HWDGE descriptor
            # FIFO that blocks the issuing sequencer.
            dma_w12_e(0, 128, nc.sync)
            dma_x0(1, nc.sync)
            dma_w12_e(128, 256, nc.sync)
            dma_w12_e(256, 512, nc.sync)
            dma_w12_e(512, 896, nc.sync)
            # scalar ring: up-half of w12 head + x0 kd2.  Its issues (and the
            # ACT silu table load) can run past the first silu's ready time;
            # LAG=3 gives the SwiGLU chain enough slack to recover.
            dma_w12_e(H, H + 128, nc.scalar)
            dma_x0(2, nc.scalar)
            dma_w12_e(H + 128, H + 256, nc.scalar)
            dma_w12_e(H + 256, H + 512, nc.scalar)
            dma_w12_e(H + 512, H + 896, nc.scalar)
            # gpsimd ring: x0 kd0/kd3, w3 in gemm2 order, then the w12 hh7-10
            # tails (SWDGE's SBUF descriptor rings don't block the issuer).
            # x blocks 1..7 are issued inside the block-0 loop, gated so
            # their transfers can't steal HBM bandwidth from the criticals.
            dma_x0(0, nc.gpsimd)
            dma_x0(3, nc.gpsimd)
            dma_w3(0, 2)
            dma_w3(2, 4)
            dma_w3(4, 7)
            dma_w3(7, KH)
            dma_w12_e(896, 1408, nc.gpsimd)
            dma_w12_e(H + 896, H + 1408, nc.gpsimd)

            for tb in range(NB):
                tsl = bass.ts(tb, NT)
                ht = hpool.tile([128, KH, NT], F16)
                acc = [
                    pacc.tile([128, NT], F32, name=f"acc{do}", tag=f"acc{do}")
                    for do in range(KD)
                ]

                def gemm2_step(kh):
                    for do in range(KD):
                        nc.tensor.matmul(
                            acc[do][:],
                            w3s[:, kh, do * 128 : (do + 1) * 128],
                            ht[:, kh, :],
                            start=(kh == 0),
                            stop=(kh == KH - 1),
                        )

                def warm_keep(hh, half):
                    # keep the PE busy across early chunk-landing stalls;
                    # targets acc[0], which gemm2's start=True clears later
                    if tb == 0:
                        for _ in range(WARM_IN.get((hh, half), 0)):
                            nc.tensor.matmul(
                                acc[0][:, 0:128], dmy[:], dmy[:],
                                start=True, stop=True,
                            )

                for hh in range(KH):
                    ps_g = pgate.tile([128, NT], F32)
                    ps_u = pup.tile([128, NT], F32)
                    for kd in range(KD):
                        nc.tensor.matmul(
                            ps_g[:],
                            w12s[:, kd, hh * 128 : (hh + 1) * 128],
                            xs[:, kd, tsl],
                            start=(kd == 0),
                            stop=(kd == KD - 1),
                        )
                    warm_keep(hh, 0)
                    for kd in range(KD):
                        nc.tensor.matmul(
                            ps_u[:],
                            w12s[:, kd, H + hh * 128 : H + (hh + 1) * 128],
                            xs[:, kd, tsl],
                            start=(kd == 0),
                            stop=(kd == KD - 1),
                        )
                    warm_keep(hh, 1)
                    sil = spool.tile([128, NT], F32)
                    nc.scalar.activation(
                        sil[:], ps_g[:], mybir.ActivationFunctionType.Silu
                    )
                    nc.vector.tensor_mul(ht[:, hh, :], sil[:], ps_u[:])
                    if tb == 0 and hh == 4:
                        # Release the x block 1..7 loads only now: a 1-elem
                        # memset creates a WAW dep that holds each transfer
                        # back until block 0's critical w12/w3 chunks have
                        # the HBM to themselves.
                        for tb2 in range(1, NB):
                            nc.vector.memset(
                                xs[0:1, 0, tb2 * NT : tb2 * NT + 1], 0.0
                            )
                            dma_x(tb2, nc.gpsimd)
                    if hh >= LAG:
                        gemm2_step(hh - LAG)
                for kh in range(KH - LAG, KH):
                    gemm2_step(kh)

                # PSUM->SBUF copies split across ACT and DVE; one coalesced
                # output DMA per block.  For the last block the epilogue is
                # the kernel's tail, so split the stores per-do across BOTH
                # HWDGE rings (SP + ACT), and halve the final do's copy and
                # store so the last dependency chain moves only 128KB.
                ot = opool.tile([128, KD, NT], F32)
                outT_r = outT[:, :].rearrange("(do p) t -> p do t", p=128)
                if tb < NB - 1:
                    for do in range(KD):
                        if do % 2 == 0:
                            nc.scalar.copy(ot[:, do, :], acc[do][:])
                        else:
                            nc.vector.tensor_copy(ot[:, do, :], acc[do][:])
                    nc.sync.dma_start(out=outT_r[:, :, tsl], in_=ot[:])
                else:
                    hn = NT // 2
                    t0, t1 = tb * NT, tb * NT + hn
                    for do in range(KD - 1):
                        if do % 2 == 0:
                            nc.scalar.copy(ot[:, do, :], acc[do][:])
                        else:
                            nc.vector.tensor_copy(ot[:, do, :], acc[do][:])
                        eng = nc.sync if do % 2 == 0 else nc.scalar
                        eng.dma_start(out=outT_r[:, do, tsl], in_=ot[:, do, :])
                    do = KD - 1
                    nc.scalar.copy(ot[:, do, 0:hn], acc[do][:, 0:hn])
                    nc.vector.tensor_copy(ot[:, do, hn:NT], acc[do][:, hn:NT])
                    nc.sync.dma_start(
                        out=outT_r[:, do, t0:t1], in_=ot[:, do, 0:hn]
                    )
                    nc.scalar.dma_start(
                        out=outT_r[:, do, t1 : (tb + 1) * NT],
                        in_=ot[:, do, hn:NT],
                    )
    nc.compile()
    return nc


_nc_cache = None


def _get_nc():
    global _nc_cache
    if _nc_cache is None:
        _nc_cache = _build()
    return _nc_cache


def kernel(sorted_x, w12, w3, expert_starts, expert_ends):
    global last_exec_time_ns, last_trace_path
    sorted_x = np.asarray(sorted_x)
    w12 = np.asarray(w12)
    w3 = np.asarray(w3)
    starts = np.asarray(expert_starts).astype(np.int64)
    T = sorted_x.shape[0]

    in_maps = []
    for e in range(N_CORES):
        # jax.lax.dynamic_slice clamps the start index the same way
        s = int(min(max(starts[e], 0), T - TPE))
        xe = sorted_x[s : s + TPE]  # (TPE, D) f32
        in_maps.append(
            {
                "xt": np.ascontiguousarray(xe.T).astype(NP_F16),
                "w12t": np.ascontiguousarray(w12[e].T).astype(NP_F16),
                "w3t": np.ascontiguousarray(w3[e].T).astype(NP_F16),
            }
        )

    trace = bool(os.environ.get("BASS_MOE_TRACE"))
    res = run_bass_kernel_spmd(
        _get_nc(), in_maps, core_ids=list(range(N_CORES)), trace=trace
    )
    if trace:
        last_exec_time_ns = res.exec_time_ns
        iat = res.instructions_and_trace
        last_trace_path = iat[1] if iat else None

    out = np.empty((N_CORES * TPE, D), dtype=np.float32)
    for e in range(N_CORES):
        out[e * TPE : (e + 1) * TPE] = res.results[e]["outT"].T
    return out



# revision 23
# speedup vs baseline: 1.0047x; 1.0047x over previous
"""Grouped SwiGLU expert MLP (MoE) on 8 Trainium2 NeuronCores.

Problem: sorted_x [32768, 512] f32, tokens pre-sorted by expert into 8 equal
contiguous segments of 4096 tokens; per-expert SwiGLU MLP
    h12 = x_e @ w12[e].T          (4096, 2816)
    h   = silu(h12[:, :1408]) * h12[:, 1408:]
    out = h @ w3[e].T             (4096, 512)

Sharding: pure expert parallelism — core e owns expert e's weights and its
4096-token segment (sliced host-side from expert_starts), so no device-side
collectives are needed; the host concatenates the per-core outputs.

Device layout is feature-major throughout ("contraction dim on partitions"),
which makes both GEMMs transpose-free on chip:
    xt   = x_e.T   [512, 4096]  fp16
    w12t = w12.T   [512, 2816]  fp16
    w3t  = w3.T    [1408, 512]  fp16
    outT = out.T   [512, 4096]  f32   (host transposes back)
GEMM1 produces H12^T tiles [128h, Nt] (PSUM), SwiGLU runs on ACT+DVE into
fp16 H^T tiles, GEMM2 consumes them directly. fp16 operands run the PE at
1 cycle/row (vs 4 for f32) — same speed and footprint as bf16 with a 10-bit
mantissa (8x lower rounding error; inputs here are well inside fp16 range).
Accumulation is always f32 in PSUM.
"""

import os

import numpy as np
import ml_dtypes

import concourse.bass as bass
import concourse.mybir as mybir
import concourse.tile as tile
from concourse import bacc
from concourse.bass_utils import run_bass_kernel_spmd

N_CORES = 8
D = 512  # d_model
H = 1408  # hidden
TWOH = 2 * H
TPE = 4096  # tokens per expert
NT = 512  # token block (matmul moving free dim, one PSUM bank in f32)
KD = D // 128  # 4 contraction tiles over d
KH = H // 128  # 11 contraction tiles over h
NB = TPE // NT  # token blocks

F16 = mybir.dt.float16
F32 = mybir.dt.float32
NP_F16 = np.dtype(np.float16)

# Results of a traced run (test harness reads these).
last_exec_time_ns = None
last_trace_path = None


def _build():
    # Bacc (not plain Bass): its compile() pass pipeline legalizes sync
    # waits (>=2 waits per instruction are split into event-sem chains),
    # which this image's walrus requires.
    nc = bacc.Bacc("TRN2", target_bir_lowering=False, debug=False, num_devices=N_CORES)
    xt = nc.dram_tensor("xt", [D, TPE], F16, kind="ExternalInput")
    w12t = nc.dram_tensor("w12t", [D, TWOH], F16, kind="ExternalInput")
    w3t = nc.dram_tensor("w3t", [H, D], F16, kind="ExternalInput")
    outT = nc.dram_tensor("outT", [D, TPE], F32, kind="ExternalOutput")

    # GEMM2 is software-pipelined into the GEMM1/SwiGLU loop with this lag:
    # in iteration hh we issue the GEMM2 matmuls consuming ht[hh - LAG], so
    # the PE never waits on the ACT+DVE SwiGLU chain (~1.3us behind).
    LAG = 3

    # Dummy warm-up matmuls issued while the first input chunks are still in
    # flight: the PE's HAM clock gate needs ~3.4us of sustained activity to
    # lift the cold 4/8 throttle, so burn the DMA head warming it up and the
    # real matmuls start at full rate.  N=128 keeps the per-dummy FIFO delay
    # (if data lands early) down to ~107ns.  25 dummies cover the ~2.7us
    # from sequencer-preamble end to the first 128KB chunks landing; a few
    # more are interleaved into block 0's first hh groups (see loop) so the
    # scattered sub-us chunk-landing stalls can't re-throttle the HAM.
    N_WARM = 44
    WARM_IN = {(0, 0): 1, (0, 1): 1, (1, 0): 1, (1, 1): 1}

    with tile.TileContext(nc) as tc:
        with (
            tc.tile_pool(name="weights", bufs=1) as wpool,
            tc.tile_pool(name="xin", bufs=1) as xpool,
            tc.tile_pool(name="ht", bufs=2) as hpool,
            tc.tile_pool(name="swi", bufs=4) as spool,
            tc.tile_pool(name="ot", bufs=4) as opool,
            tc.tile_pool(name="dmy", bufs=1) as dpool,
            tc.tile_pool(name="pg", bufs=2, space=bass.MemorySpace.PSUM) as pgate,
            tc.tile_pool(name="pu", bufs=2, space=bass.MemorySpace.PSUM) as pup,
            tc.tile_pool(name="po", bufs=1, space=bass.MemorySpace.PSUM) as pacc,
        ):
            w12s = wpool.tile([128, KD, TWOH], F16)
            w3s = wpool.tile([128, KH, D], F16)
            xs = xpool.tile([128, KD, TPE], F16)
            dmy = dpool.tile([128, 128], F16)

            nc.vector.memset(dmy[:], 0.0)
            ps_dummy = pgate.tile([128, NT], F32, tag="ps_g")
            for _ in range(N_WARM):
                nc.tensor.matmul(
                    ps_dummy[:, 0:128], dmy[:], dmy[:], start=True, stop=True
                )

            # Each DMA_DIRECT2D costs ~650ns (HWDGE) / ~900ns (SWDGE) of
            # sequencer issue time, each ring executes its transfers serially
            # in FIFO order, and the 16 SDMA engines round-robin between busy
            # rings at packet granularity (so N busy rings each get ~1/N of
            # the ~358GB/s HBM bandwidth).  Layout: the two rings each carry
            # ~half of the block-0-critical bytes, ordered EXACTLY as the PE
            # consumes them, with the non-critical x blocks 2..7 split across
            # both ring tails:
            #   SP   (qSPDynamicHW): w12 gate/up-interleaved hh-chunks, then
            #                        x2,x4,x6, then per-block output stores
            #   Pool (SWDGE):        x block 0 per-kd (the first matmul only
            #                        needs one 128KB chunk), w3 in gemm2
            #                        order, x1, then x3,x5,x7
            #   ACT  (qActDynamicHW): idle until the last block's stores
            # Deps are per-DMA-instruction, so fine chunks = early starts.
            xt_r = xt[:, :].rearrange("(kd p) t -> p kd t", p=128)
            w12_r = w12t[:, :].rearrange("(kd p) h -> p kd h", p=128)
            w3_r = w3t[:, :].rearrange("(kh p) d -> p kh d", p=128)

            def dma_w12(c0, c1):
                nc.sync.dma_start(out=w12s[:, :, c0:c1], in_=w12_r[:, :, c0:c1])

            def dma_x(tb, eng):
                eng.dma_start(
                    out=xs[:, :, tb * NT : (tb + 1) * NT],
                    in_=xt_r[:, :, tb * NT : (tb + 1) * NT],
                )

            def dma_x0(kd, eng):
                eng.dma_start(out=xs[:, kd, 0:NT], in_=xt_r[:, kd, 0:NT])

            def dma_w3(k0, k1):
                nc.gpsimd.dma_start(out=w3s[:, k0:k1, :], in_=w3_r[:, k0:k1, :])

            def dma_w12_e(c0, c1, eng):
                eng.dma_start(out=w12s[:, :, c0:c1], in_=w12_r[:, :, c0:c1])

            # Three rings each carry ~1/3 of the block-0-critical bytes in
            # their consumers' order (HWDGE rings backpressure the issuing
            # sequencer past ~9 outstanding DMAs, so each ring gets few,
            # need-ordered chunks; x tails ride the SWDGE ring, whose
            # descriptor rings absorb deep queues).
            # sync ring: gate-half of w12 head + x0 kd1, then stores.
            # 5 DMAs ~= 2.2K descriptors stays under the HWDGE descriptor
            # FIFO that blocks the issuing sequencer.
            dma_w12_e(0, 128, nc.sync)
            dma_x0(1, nc.sync)
            dma_w12_e(128, 256, nc.sync)
            dma_w12_e(256, 512, nc.sync)
            dma_w12_e(512, 896, nc.sync)
            dma_w12_e(896, 1408, nc.sync)
            # scalar ring: up-half of w12 head + x0 kd2.  Its issues (and the
            # ACT silu table load) can run past the first silu's ready time;
            # the deferred block-0 gemm2 gives the SwiGLU chain huge slack.
            dma_w12_e(H, H + 128, nc.scalar)
            dma_x0(2, nc.scalar)
            dma_w12_e(H + 128, H + 256, nc.scalar)
            dma_w12_e(H + 256, H + 512, nc.scalar)
            dma_w12_e(H + 512, H + 896, nc.scalar)
            dma_w12_e(H + 896, H + 1408, nc.scalar)
            # gpsimd ring: x0 kd0/kd3, then w3 (block 0 defers ALL its gemm2
            # until after gemm1, so w3 is not needed until ~19us into the
            # stream), then the x 1..7 tails.
            dma_x0(0, nc.gpsimd)
            dma_x0(3, nc.gpsimd)
            dma_w3(0, 4)
            dma_w3(4, KH)
            for tb in range(1, NB):
                dma_x(tb, nc.gpsimd)

            for tb in range(NB):
                # Block 0 defers ALL gemm2 until after its gemm1, so the
                # startup stream only has to keep up with w12+x0 (~150GB/s);
                # later blocks pipeline gemm2 with LAG=3.
                lag = KH if tb == 0 else LAG
                tsl = bass.ts(tb, NT)
                ht = hpool.tile([128, KH, NT], F16)
                acc = [
                    pacc.tile([128, NT], F32, name=f"acc{do}", tag=f"acc{do}")
                    for do in range(KD)
                ]

                def gemm2_step(kh):
                    for do in range(KD):
                        nc.tensor.matmul(
                            acc[do][:],
                            w3s[:, kh, do * 128 : (do + 1) * 128],
                            ht[:, kh, :],
                            start=(kh == 0),
                            stop=(kh == KH - 1),
                        )

                def warm_keep(hh, half):
                    # keep the PE busy across early chunk-landing stalls;
                    # targets acc[0], which gemm2's start=True clears later
                    if tb == 0:
                        for _ in range(WARM_IN.get((hh, half), 0)):
                            nc.tensor.matmul(
                                acc[0][:, 0:128], dmy[:], dmy[:],
                                start=True, stop=True,
                            )

                for hh in range(KH):
                    ps_g = pgate.tile([128, NT], F32)
                    ps_u = pup.tile([128, NT], F32)
                    for kd in range(KD):
                        nc.tensor.matmul(
                            ps_g[:],
                            w12s[:, kd, hh * 128 : (hh + 1) * 128],
                            xs[:, kd, tsl],
                            start=(kd == 0),
                            stop=(kd == KD - 1),
                        )
                    warm_keep(hh, 0)
                    for kd in range(KD):
                        nc.tensor.matmul(
                            ps_u[:],
                            w12s[:, kd, H + hh * 128 : H + (hh + 1) * 128],
                            xs[:, kd, tsl],
                            start=(kd == 0),
                            stop=(kd == KD - 1),
                        )
                    warm_keep(hh, 1)
                    sil = spool.tile([128, NT], F32)
                    nc.scalar.activation(
                        sil[:], ps_g[:], mybir.ActivationFunctionType.Silu
                    )
                    nc.vector.tensor_mul(ht[:, hh, :], sil[:], ps_u[:])
                    if hh >= lag:
                        gemm2_step(hh - lag)
                for kh in range(KH - lag, KH):
                    gemm2_step(kh)

                # PSUM->SBUF copies split across ACT and DVE; one coalesced
                # output DMA per block.  For the last block the epilogue is
                # the kernel's tail, so split the stores per-do across BOTH
                # HWDGE rings (SP + ACT), and halve the final do's copy and
                # store so the last dependency chain moves only 128KB.
                ot = opool.tile([128, KD, NT], F32)
                outT_r = outT[:, :].rearrange("(do p) t -> p do t", p=128)
                if tb < NB - 1:
                    for do in range(KD):
                        if do % 2 == 0:
                            nc.scalar.copy(ot[:, do, :], acc[do][:])
                        else:
                            nc.vector.tensor_copy(ot[:, do, :], acc[do][:])
                    nc.sync.dma_start(out=outT_r[:, :, tsl], in_=ot[:])
                else:
                    hn = NT // 2
                    t0, t1 = tb * NT, tb * NT + hn
                    for do in range(KD - 1):
                        if do % 2 == 0:
                            nc.scalar.copy(ot[:, do, :], acc[do][:])
                        else:
                            nc.vector.tensor_copy(ot[:, do, :], acc[do][:])
                        eng = nc.sync if do % 2 == 0 else nc.scalar
                        eng.dma_start(out=outT_r[:, do, tsl], in_=ot[:, do, :])
                    do = KD - 1
                    nc.scalar.copy(ot[:, do, 0:hn], acc[do][:, 0:hn])
                    nc.vector.tensor_copy(ot[:, do, hn:NT], acc[do][:, hn:NT])
                    nc.sync.dma_start(
                        out=outT_r[:, do, t0:t1], in_=ot[:, do, 0:hn]
                    )
                    nc.scalar.dma_start(
                        out=outT_r[:, do, t1 : (tb + 1) * NT],
                        in_=ot[:, do, hn:NT],
                    )
    nc.compile()
    return nc


_nc_cache = None


def _get_nc():
    global _nc_cache
    if _nc_cache is None:
        _nc_cache = _build()
    return _nc_cache


def kernel(sorted_x, w12, w3, expert_starts, expert_ends):
    global last_exec_time_ns, last_trace_path
    sorted_x = np.asarray(sorted_x)
    w12 = np.asarray(w12)
    w3 = np.asarray(w3)
    starts = np.asarray(expert_starts).astype(np.int64)
    T = sorted_x.shape[0]

    in_maps = []
    for e in range(N_CORES):
        # jax.lax.dynamic_slice clamps the start index the same way
        s = int(min(max(starts[e], 0), T - TPE))
        xe = sorted_x[s : s + TPE]  # (TPE, D) f32
        in_maps.append(
            {
                "xt": np.ascontiguousarray(xe.T).astype(NP_F16),
                "w12t": np.ascontiguousarray(w12[e].T).astype(NP_F16),
                "w3t": np.ascontiguousarray(w3[e].T).astype(NP_F16),
            }
        )

    trace = bool(os.environ.get("BASS_MOE_TRACE"))
    res = run_bass_kernel_spmd(
        _get_nc(), in_maps, core_ids=list(range(N_CORES)), trace=trace
    )
    if trace:
        last_exec_time_ns = res.exec_time_ns
        iat = res.instructions_and_trace
        last_trace_path = iat[1] if iat else None

    out = np.empty((N_CORES * TPE, D), dtype=np.float32)
    for e in range(N_CORES):
        out[e * TPE : (e + 1) * TPE] = res.results[e]["outT"].T
    return out



# revision 28
# speedup vs baseline: 1.0295x; 1.0247x over previous
"""Grouped SwiGLU expert MLP (MoE) on 8 Trainium2 NeuronCores.

Problem: sorted_x [32768, 512] f32, tokens pre-sorted by expert into 8 equal
contiguous segments of 4096 tokens; per-expert SwiGLU MLP
    h12 = x_e @ w12[e].T          (4096, 2816)
    h   = silu(h12[:, :1408]) * h12[:, 1408:]
    out = h @ w3[e].T             (4096, 512)

Sharding: pure expert parallelism — core e owns expert e's weights and its
4096-token segment (sliced host-side from expert_starts), so no device-side
collectives are needed; the host concatenates the per-core outputs.

Device layout is feature-major throughout ("contraction dim on partitions"),
which makes both GEMMs transpose-free on chip:
    xt   = x_e.T   [512, 4096]  fp16
    w12t = w12.T   [512, 2816]  fp16
    w3t  = w3.T    [1408, 512]  fp16
    outT = out.T   [512, 4096]  f32   (host transposes back)
GEMM1 produces H12^T tiles [128h, Nt] (PSUM), SwiGLU runs on ACT+DVE into
fp16 H^T tiles, GEMM2 consumes them directly. fp16 operands run the PE at
1 cycle/row (vs 4 for f32) — same speed and footprint as bf16 with a 10-bit
mantissa (8x lower rounding error; inputs here are well inside fp16 range).
Accumulation is always f32 in PSUM.
"""

import os

import numpy as np
import ml_dtypes

import concourse.bass as bass
import concourse.mybir as mybir
import concourse.tile as tile
from concourse import bacc
from concourse.bass_utils import run_bass_kernel_spmd

N_CORES = 8
D = 512  # d_model
H = 1408  # hidden
TWOH = 2 * H
TPE = 4096  # tokens per expert
NT = 512  # token block (matmul moving free dim, one PSUM bank in f32)
KD = D // 128  # 4 contraction tiles over d
KH = H // 128  # 11 contraction tiles over h
NB = TPE // NT  # token blocks

F16 = mybir.dt.float16
F32 = mybir.dt.float32
NP_F16 = np.dtype(np.float16)

# Results of a traced run (test harness reads these).
last_exec_time_ns = None
last_trace_path = None


def _build():
    # Bacc (not plain Bass): its compile() pass pipeline legalizes sync
    # waits (>=2 waits per instruction are split into event-sem chains),
    # which this image's walrus requires.
    nc = bacc.Bacc("TRN2", target_bir_lowering=False, debug=False, num_devices=N_CORES)
    xt = nc.dram_tensor("xt", [D, TPE], F16, kind="ExternalInput")
    w12t = nc.dram_tensor("w12t", [D, TWOH], F16, kind="ExternalInput")
    w3t = nc.dram_tensor("w3t", [H, D], F16, kind="ExternalInput")
    outT = nc.dram_tensor("outT", [D, TPE], F32, kind="ExternalOutput")

    # GEMM2 is software-pipelined into the GEMM1/SwiGLU loop with this lag:
    # in iteration hh we issue the GEMM2 matmuls consuming ht[hh - LAG], so
    # the PE never waits on the ACT+DVE SwiGLU chain (~1.3us behind).
    LAG = 2

    # Dummy warm-up matmuls issued while the first input chunks are still in
    # flight: the PE's HAM clock gate needs ~3.4us of sustained activity to
    # lift the cold 4/8 throttle, so burn the DMA head warming it up and the
    # real matmuls start at full rate.  N=128 keeps the per-dummy FIFO delay
    # (if data lands early) down to ~107ns.  25 dummies cover the ~2.7us
    # from sequencer-preamble end to the first 128KB chunks landing; a few
    # more are interleaved into block 0's first hh groups (see loop) so the
    # scattered sub-us chunk-landing stalls can't re-throttle the HAM.
    N_WARM = 30
    WARM_IN = {}

    with tile.TileContext(nc) as tc:
        with (
            tc.tile_pool(name="weights", bufs=1) as wpool,
            tc.tile_pool(name="xin", bufs=1) as xpool,
            tc.tile_pool(name="ht", bufs=2) as hpool,
            tc.tile_pool(name="swi", bufs=4) as spool,
            tc.tile_pool(name="ot", bufs=4) as opool,
            tc.tile_pool(name="dmy", bufs=1) as dpool,
            tc.tile_pool(name="pg", bufs=2, space=bass.MemorySpace.PSUM) as pgate,
            tc.tile_pool(name="pu", bufs=2, space=bass.MemorySpace.PSUM) as pup,
            tc.tile_pool(name="po", bufs=1, space=bass.MemorySpace.PSUM) as pacc,
        ):
            w12s = wpool.tile([128, KD, TWOH], F16)
            w3s = wpool.tile([128, KH, D], F16)
            xs = xpool.tile([128, KD, TPE], F16)
            dmy = dpool.tile([128, 128], F16)

            nc.vector.memset(dmy[:], 0.0)
            ps_dummy = pgate.tile([128, NT], F32, tag="ps_g")
            for _ in range(N_WARM):
                nc.tensor.matmul(
                    ps_dummy[:, 0:128], dmy[:], dmy[:], start=True, stop=True
                )

            # Each DMA_DIRECT2D costs ~650ns (HWDGE) / ~900ns (SWDGE) of
            # sequencer issue time, each ring executes its transfers serially
            # in FIFO order, and the 16 SDMA engines round-robin between busy
            # rings at packet granularity (so N busy rings each get ~1/N of
            # the ~358GB/s HBM bandwidth).  Layout: the two rings each carry
            # ~half of the block-0-critical bytes, ordered EXACTLY as the PE
            # consumes them, with the non-critical x blocks 2..7 split across
            # both ring tails:
            #   SP   (qSPDynamicHW): w12 gate/up-interleaved hh-chunks, then
            #                        x2,x4,x6, then per-block output stores
            #   Pool (SWDGE):        x block 0 per-kd (the first matmul only
            #                        needs one 128KB chunk), w3 in gemm2
            #                        order, x1, then x3,x5,x7
            #   ACT  (qActDynamicHW): idle until the last block's stores
            # Deps are per-DMA-instruction, so fine chunks = early starts.
            xt_r = xt[:, :].rearrange("(kd p) t -> p kd t", p=128)
            w12_r = w12t[:, :].rearrange("(kd p) h -> p kd h", p=128)
            w3_r = w3t[:, :].rearrange("(kh p) d -> p kh d", p=128)

            def dma_w12(c0, c1):
                nc.sync.dma_start(out=w12s[:, :, c0:c1], in_=w12_r[:, :, c0:c1])

            def dma_x(tb, eng):
                eng.dma_start(
                    out=xs[:, :, tb * NT : (tb + 1) * NT],
                    in_=xt_r[:, :, tb * NT : (tb + 1) * NT],
                )

            def dma_x0(kd, eng):
                eng.dma_start(out=xs[:, kd, 0:NT], in_=xt_r[:, kd, 0:NT])

            def dma_w3(k0, k1):
                nc.gpsimd.dma_start(out=w3s[:, k0:k1, :], in_=w3_r[:, k0:k1, :])

            def dma_w12_e(c0, c1, eng):
                eng.dma_start(out=w12s[:, :, c0:c1], in_=w12_r[:, :, c0:c1])

            # v3 layout: two transfer rings.  sync: w12 in fine
            # consumption-ordered chunks then x2/x4/x6; gpsimd: x block 0
            # per-kd, w3, x1/x3/x5/x7.  scalar ring only used for the last
            # block's stores.
            for kd in range(KD):
                nc.gpsimd.dma_start(
                    out=xs[:, kd, 0:NT], in_=xt_r[:, kd, 0:NT]
                )
            for c0, c1 in [(0, 128), (128, 256)]:
                dma_w12(c0, c1)
                dma_w12(H + c0, H + c1)
            for c0, c1 in [(256, 512), (512, 768), (768, 1024),
                           (1024, 1280), (1280, 1408)]:
                dma_w12(c0, c1)
                dma_w12(H + c0, H + c1)
            nc.gpsimd.dma_start(out=w3s[:, 0:2, :], in_=w3_r[:, 0:2, :])
            nc.gpsimd.dma_start(out=w3s[:, 2:5, :], in_=w3_r[:, 2:5, :])
            nc.gpsimd.dma_start(out=w3s[:, 5:KH, :], in_=w3_r[:, 5:KH, :])
            dma_x(1, nc.gpsimd)
            dma_x(2, nc.sync)
            dma_x(3, nc.gpsimd)
            dma_x(4, nc.sync)
            dma_x(5, nc.gpsimd)
            dma_x(6, nc.sync)
            dma_x(7, nc.gpsimd)

            for tb in range(NB):
                lag = LAG
                tsl = bass.ts(tb, NT)
                ht = hpool.tile([128, KH, NT], F16)
                acc = [
                    pacc.tile([128, NT], F32, name=f"acc{do}", tag=f"acc{do}")
                    for do in range(KD)
                ]

                def gemm2_step(kh):
                    for do in range(KD):
                        nc.tensor.matmul(
                            acc[do][:],
                            w3s[:, kh, do * 128 : (do + 1) * 128],
                            ht[:, kh, :],
                            start=(kh == 0),
                            stop=(kh == KH - 1),
                        )

                def warm_keep(hh, half):
                    # keep the PE busy across early chunk-landing stalls;
                    # targets acc[0], which gemm2's start=True clears later
                    if tb == 0:
                        for _ in range(WARM_IN.get((hh, half), 0)):
                            nc.tensor.matmul(
                                acc[0][:, 0:128], dmy[:], dmy[:],
                                start=True, stop=True,
                            )

                for hh in range(KH):
                    ps_g = pgate.tile([128, NT], F32)
                    ps_u = pup.tile([128, NT], F32)
                    for kd in range(KD):
                        nc.tensor.matmul(
                            ps_g[:],
                            w12s[:, kd, hh * 128 : (hh + 1) * 128],
                            xs[:, kd, tsl],
                            start=(kd == 0),
                            stop=(kd == KD - 1),
                        )
                    warm_keep(hh, 0)
                    for kd in range(KD):
                        nc.tensor.matmul(
                            ps_u[:],
                            w12s[:, kd, H + hh * 128 : H + (hh + 1) * 128],
                            xs[:, kd, tsl],
                            start=(kd == 0),
                            stop=(kd == KD - 1),
                        )
                    warm_keep(hh, 1)
                    sil = spool.tile([128, NT], F32)
                    nc.scalar.activation(
                        sil[:], ps_g[:], mybir.ActivationFunctionType.Silu
                    )
                    nc.vector.tensor_mul(ht[:, hh, :], sil[:], ps_u[:])
                    if hh >= lag:
                        gemm2_step(hh - lag)
                for kh in range(KH - lag, KH):
                    gemm2_step(kh)

                # PSUM->SBUF copies split across ACT and DVE; one coalesced
                # output DMA per block.  For the last block the epilogue is
                # the kernel's tail, so split the stores per-do across BOTH
                # HWDGE rings (SP + ACT), and halve the final do's copy and
                # store so the last dependency chain moves only 128KB.
                ot = opool.tile([128, KD, NT], F32)
                outT_r = outT[:, :].rearrange("(do p) t -> p do t", p=128)
                if tb < NB - 1:
                    for do in range(KD):
                        if do % 2 == 0:
                            nc.scalar.copy(ot[:, do, :], acc[do][:])
                        else:
                            nc.vector.tensor_copy(ot[:, do, :], acc[do][:])
                    nc.sync.dma_start(out=outT_r[:, :, tsl], in_=ot[:])
                else:
                    hn = NT // 2
                    t0, t1 = tb * NT, tb * NT + hn
                    for do in range(KD - 1):
                        if do % 2 == 0:
                            nc.scalar.copy(ot[:, do, :], acc[do][:])
                        else:
                            nc.vector.tensor_copy(ot[:, do, :], acc[do][:])
                        eng = nc.sync if do % 2 == 0 else nc.scalar
                        eng.dma_start(out=outT_r[:, do, tsl], in_=ot[:, do, :])
                    do = KD - 1
                    nc.scalar.copy(ot[:, do, 0:hn], acc[do][:, 0:hn])
                    nc.vector.tensor_copy(ot[:, do, hn:NT], acc[do][:, hn:NT])
                    nc.sync.dma_start(
                        out=outT_r[:, do, t0:t1], in_=ot[:, do, 0:hn]
                    )
                    nc.scalar.dma_start(
                        out=outT_r[:, do, t1 : (tb + 1) * NT],
                        in_=ot[:, do, hn:NT],
                    )
    nc.compile()
    return nc


_nc_cache = None


def _get_nc():
    global _nc_cache
    if _nc_cache is None:
        _nc_cache = _build()
    return _nc_cache


def kernel(sorted_x, w12, w3, expert_starts, expert_ends):
    global last_exec_time_ns, last_trace_path
    sorted_x = np.asarray(sorted_x)
    w12 = np.asarray(w12)
    w3 = np.asarray(w3)
    starts = np.asarray(expert_starts).astype(np.int64)
    T = sorted_x.shape[0]

    in_maps = []
    for e in range(N_CORES):
        # jax.lax.dynamic_slice clamps the start index the same way
        s = int(min(max(starts[e], 0), T - TPE))
        xe = sorted_x[s : s + TPE]  # (TPE, D) f32
        in_maps.append(
            {
                "xt": np.ascontiguousarray(xe.T).astype(NP_F16),
                "w12t": np.ascontiguousarray(w12[e].T).astype(NP_F16),
                "w3t": np.ascontiguousarray(w3[e].T).astype(NP_F16),
            }
        )

    trace = bool(os.environ.get("BASS_MOE_TRACE"))
    res = run_bass_kernel_spmd(
        _get_nc(), in_maps, core_ids=list(range(N_CORES)), trace=trace
    )
    if trace:
        last_exec_time_ns = res.exec_time_ns
        iat = res.instructions_and_trace
        last_trace_path = iat[1] if iat else None

    out = np.empty((N_CORES * TPE, D), dtype=np.float32)
    for e in range(N_CORES):
        out[e * TPE : (e + 1) * TPE] = res.results[e]["outT"].T
    return out

